# revision 36
# baseline (speedup 1.0000x reference)
"""Sparse MLA (DeepSeek-style DSA) kernel for 8 Trainium2 NeuronCores.

Strategy: token-parallel over 8 cores (64 tokens/core, all 128 heads). The
MQA latent kv_cache (8192x576) is shared across heads; sharding tokens
divides the gather 8 ways while the per-head projections are streamed in
head blocks.

v4: the v2 on-chip transpose dma_gather (SWDGE, ~4.7us of Q7 descriptor
generation per token) is replaced by 20 PE identity-transposes per token
(kvG [k,c] chunks -> kvT [c,k] chunks via PSUM, DVE copy to SBUF), freeing
the Pool engine for the one true gather (kvG rows from HBM) and dropping
the on-chip DMA payload entirely. Indices are sorted per token host-side
(softmax is k-order invariant) for HBM locality.
Softmax normalization is deferred: exp goes PSUM->SBUF bf16 on ACT with
accumulated Z into zbuf; 1/Z is folded into the output projection as a
per-partition tensor_scalar_mul on DVE. k_b weights + q_nope ride fp8
(w*8 | qn/8) to halve that stream.

Per core (64 tokens):
  1. q_lat = einsum(q_nope, k_b^T) per head -> qcT [c, h, t] (bf16, on-chip)
  2. per 2-token group: dma_gather rows kvG [k, 640] bf16; per token: 4 xbar
     transposes derive the score operand kvT [c, k] on-chip
  3. scores = qcT^T @ kvT (PE, 5 c-chunks); exp on ACT -> p2 bf16 + Z
  4. p^T via PE transpose (identity stationary)
  5. attn_lat^T [c, h] on PE (contract k, kvG slices stationary) -> accum
  6. out[t, v] per head = accum^T @ v_b^T (4-head blocks), scaled by 1/Z
"""
import numpy as np
import ml_dtypes

import concourse.bacc as bacc
import concourse.tile as tile
import concourse.mybir as mybir
from concourse.bass_utils import run_bass_kernel_spmd

bf16 = ml_dtypes.bfloat16
F32 = mybir.dt.float32
BF = mybir.dt.bfloat16
F8 = mybir.dt.float8e4
I16 = mybir.dt.int16
np_f8 = mybir.dt.np(F8)

T, H, K, S = 512, 128, 512, 8192
NOPE, ROPE, KVL = 128, 64, 512
CD = KVL + ROPE            # 576
CPAD = 640                 # padded row (640*2B % 256 == 0 for gather)
SCALE = float(CD) ** -0.5
NC = 8
TC = T // NC               # 64 tokens per core
HB = H // 4                # 32 4-head blocks (output projection)
WSCL = 8.0                 # fp8 scaling: w*8, qn/8

EXP = mybir.ActivationFunctionType.Exp
COPY = mybir.ActivationFunctionType.Copy

_BUILT = None


def _emit(tc, nc, D, out):
    cacheG = D["cacheG"].ap()
    wq4 = D["wq4"].ap()        # [32, 128, 4, 576] fp8 (w*8 512 | qn/8 64)
    vt4 = D["vt4"].ap()        # [HB, 128, 4, 4, 128]
    qr_d = D["qr"].ap()        # [128, 128, 64] (rope rows 0:64, zeros below)
    out_ap = out.ap()          # [HB, 64, 4, 128] bf16

    r512 = nc.gpsimd.to_reg(512)

    with (
        tc.tile_pool(name="persist", bufs=1) as persist,
        tc.tile_pool(name="wpool", bufs=2) as wpool,
        tc.tile_pool(name="gpool", bufs=2) as gpool,
        tc.tile_pool(name="spool", bufs=2) as spool,
        tc.tile_pool(name="psum", bufs=2, space="PSUM") as psum,
    ):
        qcT = persist.tile([128, 5, 128, TC], BF, name="qcT")
        accum = persist.tile([128, 4, TC, 128], BF, name="accum")
        idxab = persist.tile([128, TC, 32], I16, name="idxab")
        identb = persist.tile([128, 128], BF, name="identb")
        zbuf = persist.tile([128, TC], F32, name="zbuf")
        zbufb = persist.tile([128, TC], BF, name="zbufb")
        rzT = persist.tile([64, 128], F32, name="rzT")

        nc.sync.dma_start(out=identb, in_=D["identb"].ap())

        # ---- q_lat in 4-head DMA blocks, 2-head PSUM tiles (fp8 operands)
        for hb in range(32):
            wq = wpool.tile([128, 4, 576], F8, tag="wq", name="wq")
            nc.sync.dma_start(out=wq, in_=wq4[hb])
            if hb == 2:
                # first wq blocks queued; now unblock the gathers
                nc.sync.dma_start(out=idxab, in_=D["idxab"].ap())
            for hh in range(2):
                qlp = psum.tile([128, 2, 4, TC], F32, tag="aux", name="qlp")
                for hl in range(2):
                    for cb in range(4):
                        nc.tensor.matmul(
                            qlp[:, hl, cb, :],
                            wq[:, hh * 2 + hl, cb * 128:(cb + 1) * 128],
                            wq[:, hh * 2 + hl, 512:576],
                            start=True, stop=True,
                        )
                # psum [hl, cb, t] -> qcT [cb, (2 heads), t]
                qv = qlp.rearrange("p hl cb t -> p cb hl t")
                h0 = hb * 4 + hh * 2
                nc.scalar.copy(qcT[:, 0:4, h0:h0 + 2, :], qv)

        # rope plane of q_concat^T: [128r(pad), 128h, 64t] from HBM
        nc.sync.dma_start(out=qcT[:, 4, :, :], in_=qr_d)

        # ---- token loop: 32 groups of 2 tokens; one 512-idx gather per
        # token on alternating SWDGE queues so descgen(n+1) overlaps the
        # ring drain / transfer of gather(n)
        for gg in range(TC // 2):
            kvG = gpool.tile([128, 8, CPAD], BF, tag="kvG", name="kvG",
                             bufs=3)
            for tl in range(2):
                nc.gpsimd.dma_gather(
                    kvG[:, tl * 4:(tl + 1) * 4, :], cacheG,
                    idxab[:, gg * 2 + tl, :], 512, r512, CPAD,
                    queue_num=tl,
                )
            for tl in range(2):
                t = gg * 2 + tl
                # score operand kvT [c, k] via PE identity-transposes
                # (slot s holds rows s*128..s*128+127 of this token)
                kvT = gpool.tile([128, 5, 512], BF, tag="kvT", name="kvT")
                for s in range(4):
                    kvTp = psum.tile([128, 5, 128], BF, tag="kvtp",
                                     name="kvTp")
                    for cc in range(5):
                        nc.tensor.transpose(
                            kvTp[:, cc, :],
                            kvG[:, tl * 4 + s, cc * 128:(cc + 1) * 128],
                            identb,
                        )
                    nc.vector.tensor_copy(
                        kvT[:, :, s * 128:(s + 1) * 128], kvTp
                    )
                sc = psum.tile([128, 512], F32, tag="sc", name="sc")
                for cb in range(5):
                    nc.tensor.matmul(
                        sc,
                        qcT[:, cb, :, t],
                        kvT[:, cb, :],
                        start=(cb == 0), stop=(cb == 4),
                    )
                # exp PSUM -> SBUF bf16 on ACT, row-sum Z into zbuf column
                p2 = spool.tile([128, 512], BF, tag="p2", name="p2", bufs=3)
                nc.scalar.activation(
                    p2, sc, EXP, scale=SCALE, accum_out=zbuf[:, t:t + 1]
                )
                # p^T via PE transpose (4 chunks into one bf16 PSUM tile)
                pTp = psum.tile([128, 4, 128], BF, tag="aux", name="pTp")
                for kc in range(4):
                    nc.tensor.transpose(
                        pTp[:, kc, :],
                        p2[:, kc * 128:(kc + 1) * 128],
                        identb,
                    )
                pT = spool.tile([128, 4, 128], BF, tag="pT", name="pT", bufs=3)
                nc.vector.tensor_copy(pT, pTp)
                # attn_lat^T [c-chunk, h]: contract k, kvG slices stationary
                ap_ = psum.tile([128, 4, 128], F32, tag="ap_", name="ap_")
                for cb in range(4):
                    for kc in range(4):
                        nc.tensor.matmul(
                            ap_[:, cb, :],
                            kvG[:, tl * 4 + kc, cb * 128:(cb + 1) * 128],
                            pT[:, kc, :],
                            start=(kc == 0), stop=(kc == 3),
                        )
                nc.scalar.copy(accum[:, :, t, :], ap_)

        # ---- 1/Z: transpose zbuf [h, t] -> [t, h], reciprocal on DVE
        nc.vector.tensor_copy(zbufb, zbuf)
        ztp = psum.tile([64, 128], BF, tag="aux", name="ztp")
        nc.tensor.transpose(ztp, zbufb, identb)
        nc.vector.reciprocal(rzT, ztp)

        # ---- output projection in 4-head blocks, scaled by 1/Z
        for hb in range(HB):
            vtile = wpool.tile([128, 4, 4, 128], BF, tag="vt", name="vtile")
            nc.sync.dma_start(out=vtile, in_=vt4[hb])
            op4 = psum.tile([64, 4, 128], F32, tag="aux", name="op4")
            for hl in range(4):
                h = hb * 4 + hl
                for cb in range(4):
                    nc.tensor.matmul(
                        op4[:, hl, :], accum[:, cb, :, h], vtile[:, hl, cb, :],
                        start=(cb == 0), stop=(cb == 3),
                    )
            outs = spool.tile([64, 4, 128], BF, tag="outs", name="outs")
            for hl in range(4):
                h = hb * 4 + hl
                nc.vector.tensor_scalar_mul(
                    outs[:, hl, :], op4[:, hl, :], rzT[:, h:h + 1]
                )
            nc.sync.dma_start(out=out_ap[hb], in_=outs)


def build():
    nc = bacc.Bacc("TRN2", target_bir_lowering=False, debug=False,
                   num_swdge_queues=2, dynamic_dma_scratch_size=12288)
    D = {
        "cacheG": nc.dram_tensor("cacheG", [S, CPAD], BF, kind="ExternalInput"),
        "wq4": nc.dram_tensor("wq4", [32, 128, 4, 576], F8, kind="ExternalInput"),
        "vt4": nc.dram_tensor("vt4", [HB, 128, 4, 4, 128], BF, kind="ExternalInput"),
        "qr": nc.dram_tensor("qr", [128, H, TC], BF, kind="ExternalInput"),
        "idxab": nc.dram_tensor("idxab", [128, TC, 32], I16, kind="ExternalInput"),
        "identb": nc.dram_tensor("identb", [128, 128], BF, kind="ExternalInput"),
    }
    out = nc.dram_tensor("out", [HB, TC, 4, 128], BF, kind="ExternalOutput")
    with tile.TileContext(nc) as tc:
        _emit(tc, nc, D, out)
    nc.compile()
    return nc


def _wrap_idx(flat):
    # flat [n] int16 -> [16, n//16] with w[p, s] = flat[s*16 + p]
    return np.ascontiguousarray(flat.reshape(-1, 16).T)


def host_prep_shared(kv_cache, k_b_proj_trans, v_b_proj):
    cacheG = np.zeros((S, CPAD), dtype=bf16)
    cacheG[:, :CD] = kv_cache.astype(bf16)
    # vt4[hb, p, hl, cb, v] = v_b[4hb+hl, v, cb*128+p]
    vt = v_b_proj.transpose(0, 2, 1).reshape(H, 4, 128, 128)   # [h, cb, p, v]
    vt4 = np.ascontiguousarray(
        vt.reshape(HB, 4, 4, 128, 128).transpose(0, 3, 1, 2, 4)).astype(bf16)
    identb = np.eye(128, dtype=np.float32).astype(bf16)
    return {"cacheG": cacheG, "vt4": vt4, "identb": identb}


def host_prep_core(q, k_b_proj_trans, topk_indices, core):
    qc = q[core * TC:(core + 1) * TC]                      # [64, H, 192]
    # wq4[hb, p(d), hl, 0:512] = k_b[4hb+hl, k', d]^T * 8; [..., 512:576] = qnT/8
    wt = k_b_proj_trans.transpose(0, 2, 1)                 # [h, d, k']
    qn = qc[:, :, :NOPE]                                   # [64t, h, d]
    wq4 = np.empty((32, 128, 4, 576), dtype=np_f8)
    wq4[:, :, :, :512] = (
        wt.reshape(32, 4, 128, 512).transpose(0, 2, 1, 3) * WSCL
    ).astype(np_f8)
    wq4[:, :, :, 512:] = (
        qn.reshape(TC, 32, 4, 128).transpose(1, 3, 2, 0) / WSCL
    ).astype(np_f8)
    qr = qc[:, :, NOPE:].astype(bf16)                      # [64t, h, r]
    qr_dev = np.zeros((128, H, TC), dtype=bf16)
    qr_dev[:ROPE] = qr.transpose(2, 1, 0)                  # [64r, h, 64t]
    # sorted per token: softmax/attn are k-order invariant, HBM locality up
    idx = np.sort(topk_indices[core * TC:(core + 1) * TC], axis=1)
    idx = idx.astype(np.int16)                             # [64, 512]
    idxab = np.ascontiguousarray(
        idx.reshape(TC, 32, 16).transpose(2, 0, 1))        # [16, 64, 32]
    idxab_dev = np.tile(idxab, (8, 1, 1))                  # [128, 64, 32]
    return {"wq4": wq4, "qr": qr_dev, "idxab": idxab_dev}


def make_in_maps(q, kv_cache, topk_indices, k_b_proj_trans, v_b_proj):
    shared = host_prep_shared(kv_cache, k_b_proj_trans, v_b_proj)
    return [
        {**shared, **host_prep_core(q, k_b_proj_trans, topk_indices, i)}
        for i in range(NC)
    ]


def unshard(results):
    # results[i]["out"]: [HB, 64t, 4, 128v] bf16 for core i's tokens
    parts = [
        np.asarray(r["out"]).astype(np.float32)
        .transpose(1, 0, 2, 3).reshape(TC, H, 128)
        for r in results
    ]
    return np.ascontiguousarray(np.concatenate(parts, axis=0))


def kernel(q, kv_cache, topk_indices, k_b_proj_trans, v_b_proj):
    global _BUILT
    if _BUILT is None:
        _BUILT = build()
    nc = _BUILT
    in_maps = make_in_maps(q, kv_cache, topk_indices, k_b_proj_trans, v_b_proj)
    res = run_bass_kernel_spmd(nc, in_maps, core_ids=list(range(NC)))
    return unshard(res.results)
